# revision 11
# baseline (speedup 1.0000x reference)
"""MoE expert FFN (grouped GEMM) Trainium2 kernel.

Problem: inputs [W=8, E=4, C=2048, H=1024] fp32, per-expert FFN
(W1 [E,H,4F], b1, W2 [E,4F,H], b2) with tanh-approx GELU between.
out[w,e,c,:] = FFN_e(inputs[w,e,c,:]).

Sharding (expert-parallel x token-parallel, 8 cores): core c handles
expert e = c//2 and world-slice w in [0,4) or [4,8) by c%2 -> 8192
tokens per core, one expert's weights per core.

Device layout: everything is pre-transposed on the host so the
contraction dim always lands on SBUF partitions and no on-chip
transposes are needed:
  xt  [128, 8, T]    bf16   xt[p,k,t]  = X[t, k*128+p]     (X = tokens [T,1024])
  w1  [128, 8, 4096] bf16   w1[p,k,f]  = W1[k*128+p, f]
  w2  [128, 32,1024] bf16   w2[p,k,f]  = W2[k*128+p, f]
  b1  [128, 32]      f32    b1[p,m]    = b1_full[m*128+p]
  b2  [128, 8]       f32    b2[p,m]    = b2_full[m*128+p]
  out [128, 8, T]    f32    out[p,m,t] = Y[t, m*128+p]

Per 512-token chunk: GEMM1 accumulates 8 k-tiles into a PSUM bank per
dff-tile (32 of them), ACT applies bias+gelu PSUM->SBUF bf16, GEMM2
accumulates 32 k-tiles per h-tile (8), DVE adds b2 PSUM->SBUF f32,
DMA out. Both weight matrices stay SBUF-resident (128 KiB/partition).
"""

import sys
from contextlib import ExitStack

import numpy as np

for _p in ("/opt/trn_rl_repo",):
    if _p not in sys.path:
        sys.path.insert(0, _p)

import ml_dtypes

import concourse.bacc as bacc
import concourse.tile as tile
from concourse import mybir
from concourse.bass_utils import run_bass_kernel_spmd

BF16 = ml_dtypes.bfloat16

W, E, C, H = 8, 4, 2048, 1024
DFF = 4 * H
N_CORES = 8
P = 128
T = (W // 2) * C          # tokens per core = 8192
KH = H // P               # 8 k-tiles over H
KF = DFF // P             # 32 k-tiles over DFF
NCHUNK = 512
NT = T // NCHUNK          # 16 chunks

_PROG = None              # cached compiled program


def build_program():
    nc = bacc.Bacc("TRN2", target_bir_lowering=False, debug=False,
                   num_devices=N_CORES)
    # x and out are chunk-major so every chunk DMA is contiguous per
    # partition (strided transfers fan out into per-row sub-DMAs, which
    # burns HW DGE queues + semaphores and lengthens the end-of-kernel
    # semaphore-teardown barrier).
    xt_ap = nc.dram_tensor("xt", [P, NT, KH, NCHUNK], mybir.dt.bfloat16,
                           kind="ExternalInput").ap()
    # weights grouped by OUTPUT tile m (all k-slices of one m are one
    # contiguous DMA), so each m-tile's matmuls unblock independently
    w1_ap = nc.dram_tensor("w1", [P, KF, KH, P], mybir.dt.bfloat16,
                           kind="ExternalInput").ap()
    w2_ap = nc.dram_tensor("w2", [P, KH, KF, P], mybir.dt.bfloat16,
                           kind="ExternalInput").ap()
    b1_ap = nc.dram_tensor("b1", [P, KF], mybir.dt.float32,
                           kind="ExternalInput").ap()
    b2_ap = nc.dram_tensor("b2", [P, KH], mybir.dt.float32,
                           kind="ExternalInput").ap()
    out_ap = nc.dram_tensor("out", [P, NT, KH, NCHUNK], mybir.dt.float32,
                            kind="ExternalOutput").ap()

    gelu = mybir.ActivationFunctionType.Gelu_apprx_tanh

    with tile.TileContext(nc) as tc:
        with ExitStack() as ctx:
            wpool = ctx.enter_context(tc.tile_pool(name="weights", bufs=1))
            xpool = ctx.enter_context(tc.tile_pool(name="x", bufs=2))
            gpool = ctx.enter_context(tc.tile_pool(name="g", bufs=1))
            opool = ctx.enter_context(tc.tile_pool(name="o", bufs=2))
            ospool = ctx.enter_context(tc.tile_pool(name="os", bufs=2))
            ps1 = ctx.enter_context(tc.tile_pool(name="ps1", bufs=4,
                                                 space="PSUM"))
            ps2 = ctx.enter_context(tc.tile_pool(name="ps2", bufs=4,
                                                 space="PSUM"))

            w1_sb = wpool.tile([P, KF, KH, P], mybir.dt.bfloat16, tag="w1")
            w2_sb = wpool.tile([P, KH, KF, P], mybir.dt.bfloat16, tag="w2")
            b1_sb = wpool.tile([P, KF], mybir.dt.float32, tag="b1")
            b2_sb = wpool.tile([P, KH], mybir.dt.float32, tag="b2")
            # PE warmup: dummy matmuls with no DMA dependency run while
            # the first x/w1 transfers land, flipping the HAM clock gate
            # and ramping the pstate before real work starts.  memset on
            # gpsimd (idle during the preamble, DVE is not).
            warm_sb = wpool.tile([P, NCHUNK], mybir.dt.bfloat16, tag="warm")
            zero_f32 = wpool.tile([P, 1], mybir.dt.float32, tag="zf")
            dummy_act = wpool.tile([P, 8], mybir.dt.float32, tag="da")
            nc.gpsimd.memset(warm_sb[:], 0)
            nc.gpsimd.memset(zero_f32[:], 0)
            warm_ps0 = ps1.tile([P, NCHUNK], mybir.dt.float32, tag="ps1",
                                name="warm_ps0")
            nc.tensor.matmul(warm_ps0[:], lhsT=warm_sb[:, :P],
                             rhs=warm_sb[:], start=True, stop=True)
            # preload the gelu table on the ACT engine off the critical
            # path (first real activation otherwise pays ~1.3us).
            nc.scalar.activation(dummy_act[:], warm_ps0[:, 0:8], gelu,
                                 bias=zero_f32[:, 0:1], scale=1.0)
            warm_ps = ps1.tile([P, NCHUNK], mybir.dt.float32, tag="ps1",
                               name="warm_ps")
            for _ in range(3):
                nc.tensor.matmul(warm_ps[:], lhsT=warm_sb[:, :P],
                                 rhs=warm_sb[:], start=True, stop=True)

            # Two HW DGE issuing engines (SP and Activation) = two DMA
            # queues sharing HBM bandwidth.  Critical-path transfers
            # (x chunk 0 + first W1 m-tiles) go on the sync queue in need
            # order; the bulk (W1 tail, W2, biases) goes on the scalar
            # queue so it neither delays descriptor issue of the head nor
            # monopolizes its queue.  GEMM1 consumes ~1.7us/m-tile so the
            # tail groups have plenty of slack.
            x_tiles = {}
            x_tiles[0] = xpool.tile([P, KH, NCHUNK], mybir.dt.bfloat16,
                                    tag="x", name="x_sb")
            nc.sync.dma_start(x_tiles[0][:, 0, :], xt_ap[:, 0, 0, :])
            nc.sync.dma_start(w1_sb[:, 0], w1_ap[:, 0])
            nc.sync.dma_start(x_tiles[0][:, 1, :], xt_ap[:, 0, 1, :])
            nc.sync.dma_start(w1_sb[:, 1], w1_ap[:, 1])
            nc.sync.dma_start(x_tiles[0][:, 2:4, :], xt_ap[:, 0, 2:4, :])
            nc.sync.dma_start(w1_sb[:, 2:4], w1_ap[:, 2:4])
            nc.sync.dma_start(x_tiles[0][:, 4:8, :], xt_ap[:, 0, 4:8, :])
            nc.scalar.dma_start(b1_sb[:], b1_ap[:])
            nc.scalar.dma_start(b2_sb[:], b2_ap[:])
            for lo, hi in [(4, 8), (8, 16), (16, 24), (24, 32)]:
                nc.scalar.dma_start(w1_sb[:, lo:hi], w1_ap[:, lo:hi])
            for lo, hi in [(0, 2), (2, 4), (4, 6), (6, 8)]:
                nc.scalar.dma_start(w2_sb[:, lo:hi], w2_ap[:, lo:hi])

            OQ = 4                      # output m-tiles per DMA descriptor
            for c in range(NT):
                if c not in x_tiles:
                    x_tiles[c] = xpool.tile([P, KH, NCHUNK],
                                            mybir.dt.bfloat16,
                                            tag="x", name="x_sb")
                    nc.sync.dma_start(x_tiles[c][:], xt_ap[:, c])
                x_sb = x_tiles.pop(c)

                g_sb = gpool.tile([P, KF, NCHUNK], mybir.dt.bfloat16, tag="g")
                for m in range(KF):
                    pt = ps1.tile([P, NCHUNK], mybir.dt.float32, tag="ps1")
                    for k in range(KH):
                        nc.tensor.matmul(
                            pt[:],
                            lhsT=w1_sb[:, m, k, :],
                            rhs=x_sb[:, k, :],
                            start=(k == 0), stop=(k == KH - 1))
                    nc.scalar.activation(g_sb[:, m, :], pt[:], gelu,
                                         bias=b1_sb[:, m:m + 1], scale=1.0)

                last_chunk = (c == NT - 1)
                o_sb = None
                for m in range(KH):
                    pt2 = ps2.tile([P, NCHUNK], mybir.dt.float32, tag="ps2")
                    for k in range(KF):
                        nc.tensor.matmul(
                            pt2[:],
                            lhsT=w2_sb[:, m, k, :],
                            rhs=g_sb[:, k, :],
                            start=(k == 0), stop=(k == KF - 1))
                    if last_chunk:
                        # singles on the last chunk: the final transfer is
                        # on the critical path, keep it small
                        os_sb = ospool.tile([P, NCHUNK], mybir.dt.float32,
                                            tag="os")
                        nc.vector.tensor_scalar_add(os_sb[:], pt2[:],
                                                    b2_sb[:, m:m + 1])
                        nc.sync.dma_start(out_ap[:, c, m], os_sb[:])
                    else:
                        if m % OQ == 0:
                            o_sb = opool.tile([P, OQ, NCHUNK],
                                              mybir.dt.float32, tag="o")
                        nc.vector.tensor_scalar_add(o_sb[:, m % OQ, :], pt2[:],
                                                    b2_sb[:, m:m + 1])
                        if m % OQ == OQ - 1:
                            nc.sync.dma_start(
                                out_ap[:, c, m - OQ + 1:m + 1], o_sb[:])

    nc.compile()
    return nc


def _get_prog():
    global _PROG
    if _PROG is None:
        _PROG = build_program()
    return _PROG


def _shard_inputs(inputs, W1, b1, W2, b2):
    inputs = np.asarray(inputs, dtype=np.float32)
    W1 = np.asarray(W1, dtype=np.float32)
    b1 = np.asarray(b1, dtype=np.float32)
    W2 = np.asarray(W2, dtype=np.float32)
    b2 = np.asarray(b2, dtype=np.float32)
    in_maps = []
    for core in range(N_CORES):
        e = core // 2
        wlo = (core % 2) * (W // 2)
        X = np.ascontiguousarray(inputs[wlo:wlo + W // 2, e]).reshape(T, H)
        Xb = X.astype(BF16)
        # [T,H] -> [H,T] -> [KH,P,NT,NCHUNK] -> [P,NT,KH,NCHUNK]
        xt = np.ascontiguousarray(
            Xb.T.reshape(KH, P, NT, NCHUNK).transpose(1, 2, 0, 3))
        # W1[h,f], h=k*128+p, f=m*128+c -> [p, m, k, c]
        w1 = np.ascontiguousarray(
            W1[e].astype(BF16).reshape(KH, P, KF, P).transpose(1, 2, 0, 3))
        # W2[f,h], f=k*128+p, h=m*128+c -> [p, m, k, c]
        w2 = np.ascontiguousarray(
            W2[e].astype(BF16).reshape(KF, P, KH, P).transpose(1, 2, 0, 3))
        b1c = np.ascontiguousarray(b1[e].reshape(KF, P).T)
        b2c = np.ascontiguousarray(b2[e].reshape(KH, P).T)
        in_maps.append({"xt": xt, "w1": w1, "w2": w2, "b1": b1c, "b2": b2c})
    return in_maps


def _unshard(results):
    out = np.empty((W, E, C, H), dtype=np.float32)
    for core in range(N_CORES):
        e = core // 2
        wlo = (core % 2) * (W // 2)
        dev = results[core]["out"]                  # [P, NT, KH, NCHUNK]
        # [p, c, m, n] -> [c, n, m, p] = [T, H]
        Y = dev.transpose(1, 3, 2, 0).reshape(W // 2, C, H)
        out[wlo:wlo + W // 2, e] = Y
    return out


def run_sharded(in_maps, **kwargs):
    """Compile (cached) + run on cores 0-7; returns BassKernelResults."""
    nc = _get_prog()
    return run_bass_kernel_spmd(nc, in_maps, list(range(N_CORES)), **kwargs)


def kernel(inputs, W1, b1, W2, b2):
    in_maps = _shard_inputs(inputs, W1, b1, W2, b2)
    res = run_sharded(in_maps)
    return _unshard(res.results)



# revision 12
# speedup vs baseline: 1.0113x; 1.0113x over previous
"""MoE expert FFN (grouped GEMM) Trainium2 kernel.

Problem: inputs [W=8, E=4, C=2048, H=1024] fp32, per-expert FFN
(W1 [E,H,4F], b1, W2 [E,4F,H], b2) with tanh-approx GELU between.
out[w,e,c,:] = FFN_e(inputs[w,e,c,:]).

Sharding (expert-parallel x token-parallel, 8 cores): core c handles
expert e = c//2 and world-slice w in [0,4) or [4,8) by c%2 -> 8192
tokens per core, one expert's weights per core.

Device layout: everything is pre-transposed on the host so the
contraction dim always lands on SBUF partitions and no on-chip
transposes are needed:
  xt  [128, 8, T]    bf16   xt[p,k,t]  = X[t, k*128+p]     (X = tokens [T,1024])
  w1  [128, 8, 4096] bf16   w1[p,k,f]  = W1[k*128+p, f]
  w2  [128, 32,1024] bf16   w2[p,k,f]  = W2[k*128+p, f]
  b1  [128, 32]      f32    b1[p,m]    = b1_full[m*128+p]
  b2  [128, 8]       f32    b2[p,m]    = b2_full[m*128+p]
  out [128, 8, T]    f32    out[p,m,t] = Y[t, m*128+p]

Per 512-token chunk: GEMM1 accumulates 8 k-tiles into a PSUM bank per
dff-tile (32 of them), ACT applies bias+gelu PSUM->SBUF bf16, GEMM2
accumulates 32 k-tiles per h-tile (8), DVE adds b2 PSUM->SBUF f32,
DMA out. Both weight matrices stay SBUF-resident (128 KiB/partition).
"""

import sys
from contextlib import ExitStack

import numpy as np

for _p in ("/opt/trn_rl_repo",):
    if _p not in sys.path:
        sys.path.insert(0, _p)

import ml_dtypes

import concourse.bacc as bacc
import concourse.tile as tile
from concourse import mybir
from concourse.bass_utils import run_bass_kernel_spmd

BF16 = ml_dtypes.bfloat16

W, E, C, H = 8, 4, 2048, 1024
DFF = 4 * H
N_CORES = 8
P = 128
T = (W // 2) * C          # tokens per core = 8192
KH = H // P               # 8 k-tiles over H
KF = DFF // P             # 32 k-tiles over DFF
NCHUNK = 512
NT = T // NCHUNK          # 16 chunks

_PROG = None              # cached compiled program


def build_program():
    nc = bacc.Bacc("TRN2", target_bir_lowering=False, debug=False,
                   num_devices=N_CORES)
    # x and out are chunk-major so every chunk DMA is contiguous per
    # partition (strided transfers fan out into per-row sub-DMAs, which
    # burns HW DGE queues + semaphores and lengthens the end-of-kernel
    # semaphore-teardown barrier).
    xt_ap = nc.dram_tensor("xt", [P, NT, KH, NCHUNK], mybir.dt.bfloat16,
                           kind="ExternalInput").ap()
    # weights grouped by OUTPUT tile m (all k-slices of one m are one
    # contiguous DMA), so each m-tile's matmuls unblock independently
    w1_ap = nc.dram_tensor("w1", [P, KF, KH, P], mybir.dt.bfloat16,
                           kind="ExternalInput").ap()
    w2_ap = nc.dram_tensor("w2", [P, KH, KF, P], mybir.dt.bfloat16,
                           kind="ExternalInput").ap()
    b1_ap = nc.dram_tensor("b1", [P, KF], mybir.dt.float32,
                           kind="ExternalInput").ap()
    b2_ap = nc.dram_tensor("b2", [P, KH], mybir.dt.float32,
                           kind="ExternalInput").ap()
    out_ap = nc.dram_tensor("out", [P, NT, KH, NCHUNK], mybir.dt.float32,
                            kind="ExternalOutput").ap()

    gelu = mybir.ActivationFunctionType.Gelu_apprx_tanh

    with tile.TileContext(nc) as tc:
        with ExitStack() as ctx:
            wpool = ctx.enter_context(tc.tile_pool(name="weights", bufs=1))
            xpool = ctx.enter_context(tc.tile_pool(name="x", bufs=2))
            gpool = ctx.enter_context(tc.tile_pool(name="g", bufs=1))
            opool = ctx.enter_context(tc.tile_pool(name="o", bufs=2))
            ospool = ctx.enter_context(tc.tile_pool(name="os", bufs=2))
            ps1 = ctx.enter_context(tc.tile_pool(name="ps1", bufs=4,
                                                 space="PSUM"))
            ps2 = ctx.enter_context(tc.tile_pool(name="ps2", bufs=4,
                                                 space="PSUM"))

            w1_sb = wpool.tile([P, KF, KH, P], mybir.dt.bfloat16, tag="w1")
            w2_sb = wpool.tile([P, KH, KF, P], mybir.dt.bfloat16, tag="w2")
            b1_sb = wpool.tile([P, KF], mybir.dt.float32, tag="b1")
            b2_sb = wpool.tile([P, KH], mybir.dt.float32, tag="b2")
            # PE warmup: dummy matmuls with no DMA dependency run while
            # the first x/w1 transfers land, flipping the HAM clock gate
            # and ramping the pstate before real work starts.  memset on
            # gpsimd (idle during the preamble, DVE is not).
            warm_sb = wpool.tile([P, NCHUNK], mybir.dt.bfloat16, tag="warm")
            zero_f32 = wpool.tile([P, 1], mybir.dt.float32, tag="zf")
            dummy_act = wpool.tile([P, 8], mybir.dt.float32, tag="da")
            nc.gpsimd.memset(warm_sb[:], 0)
            nc.gpsimd.memset(zero_f32[:], 0)
            warm_ps0 = ps1.tile([P, NCHUNK], mybir.dt.float32, tag="ps1",
                                name="warm_ps0")
            nc.tensor.matmul(warm_ps0[:], lhsT=warm_sb[:, :P],
                             rhs=warm_sb[:], start=True, stop=True)
            # preload the gelu table on the ACT engine off the critical
            # path (first real activation otherwise pays ~1.3us).
            nc.scalar.activation(dummy_act[:], warm_ps0[:, 0:8], gelu,
                                 bias=zero_f32[:, 0:1], scale=1.0)
            warm_ps = ps1.tile([P, NCHUNK], mybir.dt.float32, tag="ps1",
                               name="warm_ps")
            for _ in range(6):
                nc.tensor.matmul(warm_ps[:], lhsT=warm_sb[:, :P],
                                 rhs=warm_sb[:], start=True, stop=True)

            # Single DMA queue (sync engine), descriptors in need order:
            # splitting across queues halves the head's share of HBM
            # bandwidth (queues round-robin), which starves chunk-0 W1
            # m-tiles.  GEMM1 consumes ~1.7us/m-tile, each 256KB m-tile
            # transfers in ~0.7us, so one queue stays ahead after m0.
            x_tiles = {}
            x_tiles[0] = xpool.tile([P, KH, NCHUNK], mybir.dt.bfloat16,
                                    tag="x", name="x_sb")
            nc.sync.dma_start(x_tiles[0][:, 0, :], xt_ap[:, 0, 0, :])
            nc.sync.dma_start(w1_sb[:, 0], w1_ap[:, 0])
            nc.sync.dma_start(x_tiles[0][:, 1:4, :], xt_ap[:, 0, 1:4, :])
            nc.sync.dma_start(w1_sb[:, 1], w1_ap[:, 1])
            nc.sync.dma_start(x_tiles[0][:, 4:8, :], xt_ap[:, 0, 4:8, :])
            nc.sync.dma_start(w1_sb[:, 2:4], w1_ap[:, 2:4])
            nc.sync.dma_start(b1_sb[:], b1_ap[:])
            nc.sync.dma_start(b2_sb[:], b2_ap[:])
            for lo, hi in [(4, 8), (8, 16), (16, 24), (24, 32)]:
                nc.sync.dma_start(w1_sb[:, lo:hi], w1_ap[:, lo:hi])
            for lo, hi in [(0, 2), (2, 4), (4, 6), (6, 8)]:
                nc.sync.dma_start(w2_sb[:, lo:hi], w2_ap[:, lo:hi])

            OQ = 4                      # output m-tiles per DMA descriptor
            for c in range(NT):
                if c not in x_tiles:
                    x_tiles[c] = xpool.tile([P, KH, NCHUNK],
                                            mybir.dt.bfloat16,
                                            tag="x", name="x_sb")
                    nc.sync.dma_start(x_tiles[c][:], xt_ap[:, c])
                x_sb = x_tiles.pop(c)

                g_sb = gpool.tile([P, KF, NCHUNK], mybir.dt.bfloat16, tag="g")
                for m in range(KF):
                    pt = ps1.tile([P, NCHUNK], mybir.dt.float32, tag="ps1")
                    for k in range(KH):
                        nc.tensor.matmul(
                            pt[:],
                            lhsT=w1_sb[:, m, k, :],
                            rhs=x_sb[:, k, :],
                            start=(k == 0), stop=(k == KH - 1))
                    nc.scalar.activation(g_sb[:, m, :], pt[:], gelu,
                                         bias=b1_sb[:, m:m + 1], scale=1.0)

                last_chunk = (c == NT - 1)
                o_sb = None
                for m in range(KH):
                    pt2 = ps2.tile([P, NCHUNK], mybir.dt.float32, tag="ps2")
                    for k in range(KF):
                        nc.tensor.matmul(
                            pt2[:],
                            lhsT=w2_sb[:, m, k, :],
                            rhs=g_sb[:, k, :],
                            start=(k == 0), stop=(k == KF - 1))
                    if last_chunk:
                        # singles on the last chunk: the final transfer is
                        # on the critical path, keep it small
                        os_sb = ospool.tile([P, NCHUNK], mybir.dt.float32,
                                            tag="os")
                        nc.vector.tensor_scalar_add(os_sb[:], pt2[:],
                                                    b2_sb[:, m:m + 1])
                        nc.sync.dma_start(out_ap[:, c, m], os_sb[:])
                    else:
                        if m % OQ == 0:
                            o_sb = opool.tile([P, OQ, NCHUNK],
                                              mybir.dt.float32, tag="o")
                        nc.vector.tensor_scalar_add(o_sb[:, m % OQ, :], pt2[:],
                                                    b2_sb[:, m:m + 1])
                        if m % OQ == OQ - 1:
                            nc.sync.dma_start(
                                out_ap[:, c, m - OQ + 1:m + 1], o_sb[:])

    nc.compile()
    return nc


def _get_prog():
    global _PROG
    if _PROG is None:
        _PROG = build_program()
    return _PROG


def _shard_inputs(inputs, W1, b1, W2, b2):
    inputs = np.asarray(inputs, dtype=np.float32)
    W1 = np.asarray(W1, dtype=np.float32)
    b1 = np.asarray(b1, dtype=np.float32)
    W2 = np.asarray(W2, dtype=np.float32)
    b2 = np.asarray(b2, dtype=np.float32)
    in_maps = []
    for core in range(N_CORES):
        e = core // 2
        wlo = (core % 2) * (W // 2)
        X = np.ascontiguousarray(inputs[wlo:wlo + W // 2, e]).reshape(T, H)
        Xb = X.astype(BF16)
        # [T,H] -> [H,T] -> [KH,P,NT,NCHUNK] -> [P,NT,KH,NCHUNK]
        xt = np.ascontiguousarray(
            Xb.T.reshape(KH, P, NT, NCHUNK).transpose(1, 2, 0, 3))
        # W1[h,f], h=k*128+p, f=m*128+c -> [p, m, k, c]
        w1 = np.ascontiguousarray(
            W1[e].astype(BF16).reshape(KH, P, KF, P).transpose(1, 2, 0, 3))
        # W2[f,h], f=k*128+p, h=m*128+c -> [p, m, k, c]
        w2 = np.ascontiguousarray(
            W2[e].astype(BF16).reshape(KF, P, KH, P).transpose(1, 2, 0, 3))
        b1c = np.ascontiguousarray(b1[e].reshape(KF, P).T)
        b2c = np.ascontiguousarray(b2[e].reshape(KH, P).T)
        in_maps.append({"xt": xt, "w1": w1, "w2": w2, "b1": b1c, "b2": b2c})
    return in_maps


def _unshard(results):
    out = np.empty((W, E, C, H), dtype=np.float32)
    for core in range(N_CORES):
        e = core // 2
        wlo = (core % 2) * (W // 2)
        dev = results[core]["out"]                  # [P, NT, KH, NCHUNK]
        # [p, c, m, n] -> [c, n, m, p] = [T, H]
        Y = dev.transpose(1, 3, 2, 0).reshape(W // 2, C, H)
        out[wlo:wlo + W // 2, e] = Y
    return out


def run_sharded(in_maps, **kwargs):
    """Compile (cached) + run on cores 0-7; returns BassKernelResults."""
    nc = _get_prog()
    return run_bass_kernel_spmd(nc, in_maps, list(range(N_CORES)), **kwargs)


def kernel(inputs, W1, b1, W2, b2):
    in_maps = _shard_inputs(inputs, W1, b1, W2, b2)
    res = run_sharded(in_maps)
    return _unshard(res.results)

